# revision 1
# baseline (speedup 1.0000x reference)
"""Trainium2 Bass kernel for nn_DeepSSM_Net (PointNet++-style SSM head).

Strategy: pure data parallel. B=128 samples sharded 16-per-core across 8
NeuronCores. Each core holds its shard's xyz planes SBUF-resident as
[128 partitions = 16 samples x 8 groups, 8192 points] and runs:
  - farthest point sampling (10 centers): per step, squared distances via
    ScalarE Square activations (matching the reference's (p-c)^2 formula),
    running-min + max-reduce fused on VectorE (tensor_tensor_reduce),
    argmax index via max_index, winner-center gather via indirect DMA.
  - ball query fused into the same distance passes: in-radius mask encoded
    as descending index codes, per-chunk top-8 via InstMax, merged per
    sample at the end.
Device outputs per core: FPS center coords [16,30] and ball-query neighbor
codes [16,80]. The tiny gather + 3-layer shared MLP + global-batch BN +
FC head (~0.003% of FLOPs, couples samples across cores through BN batch
statistics) runs on host in numpy.
"""

import numpy as np

# problem constants (hardcoded per the task contract)
B, N, S, NSAMP = 128, 65536, 10, 8
NCORES, SPC = 8, 16          # cores, samples per core
G, FPP = 8, 8192             # partition-groups per sample, points/partition
CH, NCH = 1024, 8            # chunk columns, chunks per row
R2 = float(np.float32(0.04))
BSTEP = 1025                 # bucket code stride (w in [1,1024] + empty 0)
NBUK = G * NCH               # 64 buckets per sample
WBIG = -3.0e38

_CACHE = {}


def _build_program():
    import os
    import concourse.bass as bass
    import concourse.tile as tile
    from concourse import bacc, mybir
    from concourse._compat import with_exitstack

    stage = int(os.environ.get("KERNEL_STAGE", "9"))

    F32 = mybir.dt.float32
    U16 = mybir.dt.uint16
    U32 = mybir.dt.uint32
    Alu = mybir.AluOpType
    Act = mybir.ActivationFunctionType

    nc = bacc.Bacc("TRN2", target_bir_lowering=False, debug=False,
                   num_devices=1)

    xd = nc.dram_tensor("xd", [128, FPP], F32, kind="ExternalInput")
    yd = nc.dram_tensor("yd", [128, FPP], F32, kind="ExternalInput")
    zd = nc.dram_tensor("zd", [128, FPP], F32, kind="ExternalInput")
    idxvc_d = nc.dram_tensor("idxvc", [128, CH], F32, kind="ExternalInput")
    cs0_d = nc.dram_tensor("cs0", [128, 3], F32, kind="ExternalInput")
    iota8_d = nc.dram_tensor("iota8", [SPC, G], F32, kind="ExternalInput")
    gsoff_d = nc.dram_tensor("gsoff", [SPC, G], F32, kind="ExternalInput")
    addc_d = nc.dram_tensor("addc", [SPC, NBUK * NSAMP], F32,
                            kind="ExternalInput")
    newxyz_d = nc.dram_tensor("newxyz", [SPC, 3 * S], F32,
                              kind="ExternalOutput")
    vout_d = nc.dram_tensor("vout", [SPC, NSAMP * S], F32,
                            kind="ExternalOutput")

    @with_exitstack
    def prog(ctx, tc):
        big = ctx.enter_context(tc.tile_pool(name="big", bufs=1))
        scr = ctx.enter_context(tc.tile_pool(name="scr", bufs=2))
        sml = ctx.enter_context(tc.tile_pool(name="sml", bufs=2))
        cst = ctx.enter_context(tc.tile_pool(name="cst", bufs=1))
        csp = ctx.enter_context(tc.tile_pool(name="csp", bufs=2))
        drm = ctx.enter_context(tc.tile_pool(name="drm", bufs=2,
                                             space="DRAM"))

        X = big.tile([128, FPP], F32, tag="X")
        Y = big.tile([128, FPP], F32, tag="Y")
        Z = big.tile([128, FPP], F32, tag="Z")
        dist = big.tile([128, FPP], F32, tag="dist")
        idxvc = cst.tile([128, CH], F32, tag="idxvc")
        nc.sync.dma_start(X[:], xd.ap())
        nc.sync.dma_start(Y[:], yd.ap())
        nc.sync.dma_start(Z[:], zd.ap())
        nc.sync.dma_start(idxvc[:], idxvc_d.ap())
        nc.vector.memset(dist[:], 1.0e10)

        ones8 = cst.tile([128, 8], F32, tag="ones8")
        nc.vector.memset(ones8[:], 1.0)
        ones16 = cst.tile([SPC, G], F32, tag="ones16")
        nc.vector.memset(ones16[:], 1.0)
        r2c = cst.tile([128, 1], F32, tag="r2c")
        nc.vector.memset(r2c[:], R2)
        onec = cst.tile([128, 1], F32, tag="onec")
        nc.vector.memset(onec[:], 1.0)
        zeroc = cst.tile([128, 1], F32, tag="zeroc")
        nc.vector.memset(zeroc[:], 0.0)
        iota8 = cst.tile([SPC, G], F32, tag="iota8")
        nc.sync.dma_start(iota8[:], iota8_d.ap())
        gsoff = cst.tile([SPC, G], F32, tag="gsoff")
        nc.sync.dma_start(gsoff[:], gsoff_d.ap())
        addc = cst.tile([SPC, NBUK * NSAMP], F32, tag="addc")
        nc.sync.dma_start(addc[:], addc_d.ap())

        vt8 = cst.tile([128, S * NCH * 8], F32, tag="vt8")
        nxyz = cst.tile([SPC, 3 * S], F32, tag="nxyz")
        nc.vector.memset(nxyz[:], 0.0)

        cs = csp.tile([128, 3], F32, tag="cs")
        nc.sync.dma_start(cs[:], cs0_d.ap())

        xyz_flat = [
            bass.AP(t.ap().tensor, 0, [[1, 128 * FPP], [1, 1]])
            for t in (xd, yd, zd)
        ]

        nsteps = 0 if stage == 0 else (1 if stage == 1 else S)
        for k in range(nsteps):
            last = k == S - 1
            for j in range(NCH):
                sl = slice(j * CH, (j + 1) * CH)
                A = scr.tile([128, CH], F32, tag="A")
                Bt = scr.tile([128, CH], F32, tag="B")
                Ct = scr.tile([128, CH], F32, tag="C")
                nc.scalar.activation(A[:], X[:, sl], Act.Square,
                                     bias=cs[:, 0:1])
                nc.scalar.activation(Bt[:], Y[:, sl], Act.Square,
                                     bias=cs[:, 1:2])
                nc.scalar.activation(Ct[:], Z[:, sl], Act.Square,
                                     bias=cs[:, 2:3])
                nc.vector.tensor_add(A[:], A[:], Bt[:])
                nc.vector.tensor_add(A[:], A[:], Ct[:])
                # ball-query mask codes for center k, chunk j:
                # SG = sign(R2 - d) in {-1,0,1}; V = SG * w-codes
                SG = scr.tile([128, CH], F32, tag="SG")
                nc.scalar.activation(SG[:], A[:], Act.Sign,
                                     bias=r2c[:, 0:1], scale=-1.0)
                V = scr.tile([128, CH], F32, tag="V")
                nc.vector.tensor_mul(V[:], SG[:], idxvc[:])
                c0 = (k * NCH + j) * 8
                nc.vector.max(vt8[:, c0:c0 + 8], V[:])
                if not last:
                    nc.vector.tensor_tensor(dist[:, sl], A[:], dist[:, sl],
                                            Alu.min)
            if last:
                break
            if stage <= 2:
                continue
            # ---- argmax + next-center chain ----
            rmax = sml.tile([128, 1], F32, tag="rmax")
            nc.vector.reduce_max(rmax[:], dist[:], axis=mybir.AxisListType.X)
            rmax8 = sml.tile([128, 8], F32, tag="rmax8")
            nc.vector.tensor_scalar(rmax8[:], ones8[:], rmax[:, 0:1], None,
                                    op0=Alu.mult)
            idx8 = sml.tile([128, 8], U16, tag="idx8")
            nc.vector.max_index(idx8[:], rmax8[:], dist[:])
            pack = sml.tile([128, 2], F32, tag="pack")
            nc.vector.tensor_copy(pack[:, 0:1], rmax[:])
            nc.vector.tensor_copy(pack[:, 1:2], idx8[:, 0:1])
            dpk = drm.tile([128, 2], F32, tag="dpk")
            nc.sync.dma_start(dpk[:], pack[:])
            packT = sml.tile([SPC, 16], F32, tag="packT")
            nc.sync.dma_start(packT[:],
                              dpk.rearrange("(s g) c -> s (g c)", g=G))
            packTv = packT.rearrange("s (g c) -> s g c", c=2)
            vals = packTv[:, :, 0]
            idxs = packTv[:, :, 1]
            top8 = sml.tile([SPC, 8], F32, tag="top8")
            nc.vector.max(top8[:], vals)
            g8 = sml.tile([SPC, 8], U16, tag="g8")
            nc.vector.max_index(g8[:], top8[:], vals)
            g8f = sml.tile([SPC, 1], F32, tag="g8f")
            nc.vector.tensor_copy(g8f[:], g8[:, 0:1])
            # one-hot of winning group: relu(1 - (g* - iota)^2)
            sq8 = sml.tile([SPC, G], F32, tag="sq8")
            nc.scalar.activation(sq8[:], iota8[:], Act.Square,
                                 bias=g8f[:, 0:1], scale=-1.0)
            oh = sml.tile([SPC, G], F32, tag="oh")
            nc.scalar.activation(oh[:], sq8[:], Act.Relu,
                                 bias=onec[0:SPC, 0:1], scale=-1.0)
            pre = sml.tile([SPC, G], F32, tag="pre")
            nc.vector.tensor_add(pre[:], idxs, gsoff[:])
            ohp = sml.tile([SPC, G], F32, tag="ohp")
            nc.vector.tensor_mul(ohp[:], oh[:], pre[:])
            gidxf = sml.tile([SPC, 1], F32, tag="gidxf")
            nc.vector.reduce_sum(gidxf[:], ohp[:], axis=mybir.AxisListType.X)
            giu = sml.tile([SPC, 1], U32, tag="giu")
            nc.vector.tensor_copy(giu[:], gidxf[:])
            col = 3 * (k + 1)
            if stage >= 4:
                for c, flat in enumerate(xyz_flat):
                    nc.gpsimd.indirect_dma_start(
                        nxyz[:, col + c:col + c + 1], None, flat,
                        bass.IndirectOffsetOnAxis(ap=giu[:], axis=0))
            # broadcast -c to [128,3] via DRAM bounce
            bc = sml.tile([SPC, 3 * G], F32, tag="bc")
            bcv = bc.rearrange("s (g c) -> s g c", c=3)
            for c in range(3):
                nc.vector.tensor_scalar(
                    bcv[:, :, c], ones16[:], nxyz[:, col + c:col + c + 1],
                    -1.0, op0=Alu.mult, op1=Alu.mult)
            dbc = drm.tile([SPC, 3 * G], F32, tag="dbc")
            nc.sync.dma_start(dbc[:], bc[:])
            cs = csp.tile([128, 3], F32, tag="cs")
            nc.sync.dma_start(cs[:],
                              dbc.rearrange("s (g c) -> (s g) c", c=3))

        # ---- ball-query merge ----
        vout = cst.tile([SPC, NSAMP * S], F32, tag="vout")
        if stage < 5:
            nc.vector.memset(vout[:], 0.0)
            nc.sync.dma_start(newxyz_d.ap(), nxyz[:])
            nc.sync.dma_start(vout_d.ap(), vout[:])
            return
        dvt = drm.tile([128, S * NCH * 8], F32, tag="dvt")
        nc.sync.dma_start(dvt[:], vt8[:])
        dvtv = dvt.rearrange("(s g) (k q) -> s g k q", g=G, k=S)
        QW = NCH * 8
        for k in range(S):
            wk = sml.tile([SPC, G * QW], F32, tag="wk")
            nc.sync.dma_start(wk[:], dvtv[:, :, k, :])
            sg = sml.tile([SPC, G * QW], F32, tag="sg")
            nc.scalar.activation(sg[:], wk[:], Act.Sign,
                                 bias=zeroc[0:SPC, 0:1])
            nc.vector.tensor_scalar_max(sg[:], sg[:], 0.0)
            wc = sml.tile([SPC, G * QW], F32, tag="wc")
            nc.vector.tensor_add(wc[:], wk[:], addc[:])
            u = sml.tile([SPC, G * QW], F32, tag="u")
            nc.vector.tensor_mul(u[:], sg[:], wc[:])
            nc.vector.max(vout[:, k * 8:(k + 1) * 8], u[:])
        nc.sync.dma_start(newxyz_d.ap(), nxyz[:])
        nc.sync.dma_start(vout_d.ap(), vout[:])

    with tile.TileContext(nc) as tc:
        prog(tc)
    nc.compile()
    return nc


def _get_nc():
    if "nc" not in _CACHE:
        _CACHE["nc"] = _build_program()
    return _CACHE["nc"]


def _make_consts():
    idxvc = np.broadcast_to(
        (CH - np.arange(CH, dtype=np.float32))[None, :], (128, CH)).copy()
    iota8 = np.broadcast_to(
        np.arange(G, dtype=np.float32)[None, :], (SPC, G)).copy()
    s_idx = np.arange(SPC, dtype=np.float32)
    gsoff = (s_idx[:, None] * (N) +
             np.arange(G, dtype=np.float32)[None, :] * FPP).astype(np.float32)
    cols = np.arange(NBUK * NSAMP)
    addc = np.broadcast_to(
        ((NBUK - 1 - cols // NSAMP) * BSTEP).astype(np.float32)[None, :],
        (SPC, NBUK * NSAMP)).copy()
    return idxvc, iota8, gsoff, addc


def _make_in_maps(pc):
    idxvc, iota8, gsoff, addc = _make_consts()
    in_maps = []
    for i in range(NCORES):
        shard = pc[i * SPC:(i + 1) * SPC]          # [16, 3, 65536]
        planes = [np.ascontiguousarray(
            shard[:, c, :].reshape(128, FPP)) for c in range(3)]
        p0 = shard[:, :, 0]                        # [16, 3]
        cs0 = np.repeat(-p0, G, axis=0).astype(np.float32)   # [128, 3]
        in_maps.append({
            "xd": planes[0], "yd": planes[1], "zd": planes[2],
            "idxvc": idxvc, "cs0": cs0, "iota8": iota8,
            "gsoff": gsoff, "addc": addc,
        })
    return in_maps


def _decode_neighbors(vout):
    """vout: [B, S, 8] merged codes -> idx [B, S, 8] int32 (reference
    semantics: first 8 in-radius points by index, padded with slot 0)."""
    u = vout.astype(np.int64)
    buck = (NBUK - 1) - u // BSTEP
    w = u % BSTEP
    n = buck * CH + (CH - w)
    empty = u == 0
    n = np.where(empty, n[:, :, 0:1], n)
    return n.astype(np.int32)


def _host_head(pc, new_xyz, idx, p):
    """grouping + shared MLP + BN + FC head (numpy, float64 accum)."""
    xyz = pc.transpose(0, 2, 1).astype(np.float64)       # [B, N, 3]
    bi = np.arange(B)[:, None, None]
    grouped = xyz[bi, idx]                               # [B, S, 8, 3]
    grouped = grouped - new_xyz[:, :, None, :].astype(np.float64)
    x = grouped.transpose(0, 3, 2, 1)                    # [B, 3, 8, S]

    def bn(v, g, be):
        m = v.mean(axis=(0, 2, 3), keepdims=True)
        var = v.var(axis=(0, 2, 3), keepdims=True)
        return (v - m) / np.sqrt(var + 1e-5) * g[None, :, None, None] \
            + be[None, :, None, None]

    for w, b, g, be in (("w1", "b1", "g1", "be1"), ("w2", "b2", "g2", "be2"),
                        ("w3", "b3", "g3", "be3")):
        w, b, g, be = (p[w].astype(np.float64), p[b].astype(np.float64),
                       p[g].astype(np.float64), p[be].astype(np.float64))
        x = np.einsum("oc,bcns->bons", w, x) + b[None, :, None, None]
        x = np.maximum(bn(x, g, be), 0.0)
    x = x.max(axis=2)                                    # [B, 16, S]
    feat = x.reshape(B, -1)
    h = feat @ p["fc1_w"].astype(np.float64).T + p["fc1_b"].astype(np.float64)
    m = h.mean(0, keepdims=True)
    v = h.var(0, keepdims=True)
    h = (h - m) / np.sqrt(v + 1e-5) * p["bn1_g"].astype(np.float64) \
        + p["bn1_b"].astype(np.float64)
    h = np.maximum(h, 0.0)
    out = h @ p["fc2_w"].astype(np.float64).T + p["fc2_b"].astype(np.float64)
    return out.astype(np.float32)


def run_device(pc, trace=False, return_raw=False):
    """Returns (new_xyz [B,S,3] f32, idx [B,S,8] i32) from the 8-core run."""
    from concourse import bass_utils
    nc = _get_nc()
    in_maps = _make_in_maps(pc)
    res = bass_utils.run_bass_kernel_spmd(nc, in_maps,
                                          core_ids=list(range(NCORES)),
                                          trace=trace)
    new_xyz = np.zeros((B, S, 3), np.float32)
    vout = np.zeros((B, S, NSAMP), np.float32)
    for i in range(NCORES):
        r = res.results[i]
        new_xyz[i * SPC:(i + 1) * SPC] = r["newxyz"].reshape(SPC, S, 3)
        vout[i * SPC:(i + 1) * SPC] = r["vout"].reshape(SPC, S, NSAMP)
    # slot 0 of step 0 center comes from host (point 0 of each sample)
    new_xyz[:, 0, :] = pc[:, :, 0]
    idx = _decode_neighbors(vout)
    if return_raw:
        return new_xyz, idx, res
    return new_xyz, idx


def kernel(**inputs):
    pc = np.ascontiguousarray(inputs["pc_electrode"], dtype=np.float32)
    new_xyz, idx = run_device(pc)
    return _host_head(pc, new_xyz, idx, inputs)

